# revision 2
# baseline (speedup 1.0000x reference)
"""Trainium2 Bass kernel for MViTv2-style attention (decomposed rel-pos bias).

Problem: B=8, H=W=32, DIM=768, NH=12, HD=64, S=1024.
Sharding: data-parallel, one batch element per NeuronCore (8 cores).

v2 redesign vs baseline (450us):
  - all DRAM tensors bf16 (16.3MB -> ~6.6MB HBM), matmuls in bf16
  - loads interleaved with q-projection matmuls (per-dt gating)
  - one ACT bias-copy per q/k group ([128,512]); odd head staged in rows
    64:128 of the even head's qaug slot, then ONE sbuf-sbuf hop DMA
  - rel-pos matmuls land on partitions 64:96 / 96:128 directly via
    col-tiled tile_position -> no partition-shift DMAs, plain copies
  - phase 2: exp reads [128,1024] across 2 psum banks (4 ACT ops/iter);
    softmax denominator chain fully on DVE (reciprocal_approx_fast);
    v-projection matmuls deferred into phase 2 to keep PE busy (HAM)
  - phase 3: paired [128,1024] ACT copies, bf16 output
"""
import numpy as np

B, H, W, DIM, NH = 8, 32, 32, 768, 12
HD = DIM // NH          # 64
S = H * W               # 1024
SCALE = HD ** -0.5
NCORES = 8


N_ITERS = 24


def _v_sched():
    """v-group ids per phase-2 iteration: 24 deferred groups (hp 3..5)."""
    per_iter = [2] * 4 + [1] * 16 + [0] * 4
    sched, gv = [], 24
    for n in per_iter:
        sched.append(list(range(gv, gv + n)))
        gv += n
    assert gv == 48
    return sched


def _phase1b_events():
    """PE-order event plan for phase 1B: rel pairs + k groups, then v."""
    events = []
    for j in range(32):
        events.append(("rel", j))
        if j % 2 == 1 and (j - 1) // 2 < 12:
            events.append(("k", 12 + (j - 1) // 2))
    for gv in range(24):
        events.append(("v", gv))
    return events


def build_nc():
    import concourse.bass as bass
    import concourse.mybir as mybir
    from contextlib import ExitStack

    F32 = mybir.dt.float32
    F32R = mybir.dt.float32r
    BF16 = mybir.dt.bfloat16
    I32 = mybir.dt.int32
    AF = mybir.ActivationFunctionType
    ALU = mybir.AluOpType

    nc = bass.Bass(detect_race_conditions=False)

    # ---- DRAM parameters (per core) ----
    # host pre-shuffles (dt p) -> p-major so each partition is one
    # contiguous DRAM run (128 big DMA descriptors instead of 768 small)
    xT_e = nc.declare_dram_parameter("xT", [128, 6 * S], BF16, isOutput=False)
    wqk_e = nc.declare_dram_parameter("wqk", [128, 2 * 6 * DIM], BF16, isOutput=False)
    wv_e = nc.declare_dram_parameter("wv", [128, 6 * DIM], BF16, isOutput=False)
    wproj_e = nc.declare_dram_parameter("wproj", [128, 6 * DIM], BF16, isOutput=False)
    relh_e = nc.declare_dram_parameter("relh", [HD, 35 * 32], F32R, isOutput=False)
    relw_e = nc.declare_dram_parameter("relw", [HD, 35 * 32], F32R, isOutput=False)
    oneh_e = nc.declare_dram_parameter("onehot", [HD, S], F32R, isOutput=False)
    onescol_e = nc.declare_dram_parameter("onescol", [128, 6 * 64], BF16, isOutput=False)
    qkb_e = nc.declare_dram_parameter("qkb", [128, 12], F32, isOutput=False)
    projb_e = nc.declare_dram_parameter("projb", [128, 6], F32, isOutput=False)
    outT_e = nc.declare_dram_parameter("outT", [DIM, S], BF16, isOutput=True)

    events1b = _phase1b_events()
    v_sched = _v_sched()

    ctx = ExitStack()
    with ctx:
        # ---- SBUF ----
        # f32r for qaug/kaug: bf16 strided writes run ~3.3x slow on ACT/DVE
        # (the rel-w copies scatter into qaug with stride 32) and walrus
        # rejects mixed 32/16-bit matmul inputs, so both aug tensors are
        # 4-byte. To fit 192KB, phase-1-only tensors (wA/wB/oneh/kscr/rel
        # tables) share their region with tensors first written in phase 2+
        # (exp_sb/ttscr/seed/t/recip/bcast/out_sb) via stack scoping; the
        # phase-2 gates guarantee all reuse is ordered at runtime.
        xT = ctx.enter_context(nc.sbuf_tensor("xT_sb", [128, 6, S], BF16))
        wv_sb = ctx.enter_context(nc.sbuf_tensor("wv_sb", [128, 6, DIM], BF16))
        wproj = ctx.enter_context(nc.sbuf_tensor("wproj_sb", [128, 6, DIM], BF16))
        qaug = ctx.enter_context(nc.sbuf_tensor("qaug", [128, NH, S], F32R))
        kaug = ctx.enter_context(nc.sbuf_tensor("kaug", [128, NH, S], F32R))
        # vaug per tt: [v0|ones|v1  v2|ones|v3 ...] (6 pairs x 192); head m
        # lhsT = cols (m//2)*192 + (64 if m odd else 0), width 128; the ones
        # block doubles as the softmax-denominator column bundle, so PV
        # output rows split into v-rows and 64 denominator rows by parity
        vaug = ctx.enter_context(nc.sbuf_tensor("vaug", [128, 8, 1152], BF16))
        outdT = ctx.enter_context(nc.sbuf_tensor("outdT_sb", [128, 6, S], BF16))
        krow = ctx.enter_context(nc.sbuf_tensor("krow", [128, 512], I32))
        qkb_sb = ctx.enter_context(nc.sbuf_tensor("qkb_sb", [128, 12], F32))
        projb_sb = ctx.enter_context(nc.sbuf_tensor("projb_sb", [128, 6], F32))
        dummy = ctx.enter_context(nc.sbuf_tensor("expwarm", [1, 8], F32))
        warm = ctx.enter_context(nc.sbuf_tensor("warm", [1, 512], BF16))
        # phase-1-only scope (freed below; region reused by phase-2 tensors)
        p1 = ExitStack()
        wA = p1.enter_context(nc.sbuf_tensor("wA", [128, 6, DIM], BF16))
        wB = p1.enter_context(nc.sbuf_tensor("wB", [128, 6, DIM], BF16))
        oneh_sb = p1.enter_context(nc.sbuf_tensor("oneh_sb", [64, S], F32R))
        relh = p1.enter_context(nc.sbuf_tensor("relh_sb", [HD, 35 * 32], F32R))
        relw = p1.enter_context(nc.sbuf_tensor("relw_sb", [HD, 35 * 32], F32R))
        kscr = [p1.enter_context(
            nc.sbuf_tensor(f"kscr{o}", [128, 512], F32R)) for o in range(2)]
        p1.close()
        # phase-2+ tensors, overlapping the freed phase-1 region
        exp_sb = ctx.enter_context(nc.sbuf_tensor("exp_sb", [128, 2, 8 * 512], BF16))
        seed_sb = ctx.enter_context(nc.sbuf_tensor("seed_sb", [128, 512], I32))
        t_sb = ctx.enter_context(nc.sbuf_tensor("t_sb", [128, 512], F32))
        recip_sb = ctx.enter_context(nc.sbuf_tensor("recip_sb", [128, 512], F32))
        out_sb = [ctx.enter_context(
            nc.sbuf_tensor(f"out_sb{o}", [128, 1024], BF16)) for o in range(2)]

        # ---- PSUM (8 banks) ----
        qkA = ctx.enter_context(nc.psum_tensor("qkA", [128, 1024], F32))
        qkB = ctx.enter_context(nc.psum_tensor("qkB", [128, 1024], F32))
        out_ps = [ctx.enter_context(nc.psum_tensor(f"out_ps{o}", [128, 512], F32))
                  for o in range(2)]
        # bc_ps rows 0:64 hold the bcast result; odd v-groups borrow the bank
        bc_ps = ctx.enter_context(nc.psum_tensor("bc_ps", [128, 512], F32))
        v_ps = ctx.enter_context(nc.psum_tensor("v_ps", [128, 512], F32))

        banks4 = [qkA[:, 0:512], qkA[:, 512:1024],
                  qkB[:, 0:512], qkB[:, 512:1024]]
        qk2 = [qkA, qkB]

        # ---- semaphores ----
        s_ld1 = ctx.enter_context(nc.semaphore("s_ld1"))  # xT + wA loads
        s_ld2 = ctx.enter_context(nc.semaphore("s_ld2"))  # wB loads
        s_ld3 = ctx.enter_context(nc.semaphore("s_ld3"))  # wproj loads
        s_ls = ctx.enter_context(nc.semaphore("s_ls"))    # rel tables (gpsimd)
        s_lc = ctx.enter_context(nc.semaphore("s_lc"))    # tiny consts (scalar)
        s_lw = ctx.enter_context(nc.semaphore("s_lw"))    # wv loads (scalar q)
        s_lg = ctx.enter_context(nc.semaphore("s_lg"))    # onescol (gpsimd)
        s_qh = ctx.enter_context(nc.semaphore("s_qh"))    # q odd-head hops
        s_kh = [ctx.enter_context(nc.semaphore(f"s_kh{o}"))
                for o in range(2)]                        # k hops per kscr slot
        s_oh = ctx.enter_context(nc.semaphore("s_oh"))    # onehot repl (ACT ring)
        s_oh2 = ctx.enter_context(nc.semaphore("s_oh2"))  # onehot repl (gpsimd)
        s_out0 = ctx.enter_context(nc.semaphore("s_out0"))
        s_out1 = ctx.enter_context(nc.semaphore("s_out1"))
        s_pe = ctx.enter_context(nc.semaphore("s_pe"))
        s_act = ctx.enter_context(nc.semaphore("s_act"))
        s_dve = ctx.enter_context(nc.semaphore("s_dve"))
        s_init = ctx.enter_context(nc.semaphore("s_init"))  # DVE memset init
        s_vpe = ctx.enter_context(nc.semaphore("s_vpe"))    # v matmul groups
        s_vdve = ctx.enter_context(nc.semaphore("s_vdve"))  # v psum->vaug copies

        block = ctx.enter_context(nc.Block())

        # ---- analytic semaphore layout ----
        # phase 1: s_pe: q groups 1..12, then events1b: k groups / rel mms
        # (2 per pair); s_act: q copies 1..12, then per events1b (k copy 1,
        # rel-even pair 2); s_dve: rel-odd pairs (2 each) = 32 total
        pe_n = 12
        act_n = 12
        dve_n = 0
        mark_pe_k = {}        # k group -> s_pe after its stop mm
        mark_pe_rel = {}      # rel pair -> s_pe after both mms
        mark_act_grp = {}     # q/k group -> s_act after its copy
        for g in range(12):
            mark_act_grp[g] = g + 1
        mark_act_rel = {}     # even rel pair -> s_act after 2 copies
        mark_dve_rel = {}     # odd rel pair -> s_dve after 2 copies
        for ev, arg in events1b:
            if ev == "rel":
                pe_n += 2
                mark_pe_rel[arg] = pe_n
                if arg % 2 == 0:
                    act_n += 2
                    mark_act_rel[arg] = act_n
                else:
                    dve_n += 2
                    mark_dve_rel[arg] = dve_n
            elif ev == "k":
                pe_n += 1
                mark_pe_k[arg] = pe_n
                act_n += 1
                mark_act_grp[arg] = act_n
        PE0, ACT0, DVE0 = pe_n, act_n, dve_n    # 88, 56, 32
        assert (PE0, ACT0, DVE0) == (88, 56, 32)

        def rel_banks(j):
            # (h-mm bank, w-mm bank) for rel pair j; two bank-sets, depth-2
            return ([out_ps[0], out_ps[1]], [bc_ps, v_ps])[j % 2]

        def rel_prev(j):
            # previous rel pair that used the same bank set
            return j - 2 if j >= 2 else None

        def pe_base(it):
            return PE0 + 16 * it

        def pv7_inc(j):
            return pe_base(j + 1) + 12 if j < N_ITERS - 1 else PE0 + 16 * N_ITERS + 4

        iters = [(m, b) for m in range(NH) for b in range(2)]

        # ================= sync queue: loads, hops, out =================
        def _sync(sync):
            sync.dma_start(out=xT[:, 0:3, :], in_=xT_e[:, 0:3 * S]
                           ).then_inc(s_ld1, 16)
            sync.dma_start(out=wA[:, 0:3, :], in_=wqk_e[:, 0:3 * DIM]
                           ).then_inc(s_ld1, 16)
            sync.dma_start(out=wB[:], in_=wqk_e[:, 6 * DIM:12 * DIM]
                           ).then_inc(s_ld2, 16)
            sync.dma_start(out=wproj[:], in_=wproj_e[:]).then_inc(s_ld3, 16)
            # q odd-head hops: qaug[64:128, 2jt] -> qaug[0:64, 2jt+1],
            # one full-row DMA per head pair (waits both halves' copies)
            for jt in range(6):
                sync.wait_ge(s_act, mark_act_grp[2 * jt + 1])
                sync.dma_start(
                    out=qaug[0:64, 2 * jt + 1, :],
                    in_=qaug[64:128, 2 * jt, :],
                ).then_inc(s_qh, 16)
            # k hops: kscr[g%2] rows 0:64 -> kaug[0:64, 2jt] (even head),
            # rows 64:128 -> kaug[0:64, 2jt+1] (odd head)
            for g in range(12, 24):
                jt, b = (g - 12) // 2, (g - 12) % 2
                sync.wait_ge(s_act, mark_act_grp[g])
                sync.dma_start(
                    out=kaug[0:64, 2 * jt, b * 512:(b + 1) * 512],
                    in_=kscr[g % 2][0:64, :],
                ).then_inc(s_kh[g % 2], 16)
                sync.dma_start(
                    out=kaug[0:64, 2 * jt + 1, b * 512:(b + 1) * 512],
                    in_=kscr[g % 2][64:128, :],
                ).then_inc(s_kh[g % 2], 16)
            # phase-3 output stores
            for jt in range(6):
                sync.wait_ge(s_act, ACT0 + 96 + jt + 1)
                sync.dma_start(
                    out=outT_e[jt * 128:(jt + 1) * 128, :],
                    in_=out_sb[jt % 2][:, 0:1024],
                ).then_inc([s_out0, s_out1][jt % 2], 16)
            sync.wait_ge(s_out0, 3 * 16)
            sync.wait_ge(s_out1, 3 * 16)

        block.sync(_sync)

        # ================= gpsimd queue: onescol + onehot repl =================
        def _gp(gp):
            gp.dma_start(out=relh[:], in_=relh_e[:]).then_inc(s_ls, 16)
            gp.dma_start(out=relw[:], in_=relw_e[:]).then_inc(s_ls, 16)
            gp.dma_start(out=oneh_sb[:], in_=oneh_e[:]).then_inc(s_ls, 16)
            va6 = vaug[:].rearrange("p t (hp blk) -> p t hp blk", blk=192)
            with nc.allow_non_contiguous_dma(reason="64-col ones blocks"):
                for sk in range(8):
                    gp.dma_start(out=va6[:, sk, :, 64:128],
                                 in_=onescol_e[:].rearrange(
                                     "p (hp c) -> p hp c", c=64),
                                 ).then_inc(s_lg, 16)
            # onehot rows into kaug[64:128, m] for heads 6..11 (0..5 go
            # via the scalar ring; kaug upper rows have no other writers)
            # FIFO after the oneh load on this same ring -- no wait needed
            for mm_ in range(10, NH):
                gp.dma_start(out=kaug[64:128, mm_, :], in_=oneh_sb[:],
                             ).then_inc(s_oh2, 16)

        block.gpsimd(_gp)

        # ================= PE =================
        def _pe(tensor):
            C = 0
            # warm up the PE clock (HAM ramps ~3.4us) while loads stream
            tensor.wait_ge(s_init, 1)
            for _ in range(10):
                tensor.matmul(
                    bc_ps[0:64, :], warm[0:1, 0:64], warm[0:1, :],
                    start=True, stop=True,
                )
            # --- phase 1A: q then (in events) k projections ---
            for g in range(12):
                jt, b = g // 2, g % 2
                if g >= 4:
                    tensor.wait_ge(s_act, mark_act_grp[g - 4])
                if g == 0:
                    tensor.wait_ge(s_ld1, 4 * 16)
                for dt in range(6):
                    mm = tensor.matmul(
                        banks4[g % 4],
                        wA[:, dt, jt * 128:(jt + 1) * 128],
                        xT[:, dt, b * 512:(b + 1) * 512],
                        start=(dt == 0), stop=(dt == 5),
                    )
                mm.then_inc(s_pe, 1)
                C += 1
                assert C == g + 1
            # --- phase 1B: rel pairs + k groups + v prefill (planned) ---
            qaug4 = qaug[:].rearrange("p m (h w) -> p m h w", w=32)
            first_rel = True
            first_k = True
            first_v = True
            for ev, arg in events1b:
                if ev == "rel":
                    j = arg
                    if first_rel:
                        tensor.wait_ge(s_qh, 6 * 16)   # all q hops
                        tensor.wait_ge(s_ls, 3 * 16)    # relh/relw loaded
                        first_rel = False
                    jp = rel_prev(j)
                    if jp is not None:
                        if jp % 2 == 0:
                            tensor.wait_ge(s_act, mark_act_rel[jp])
                        else:
                            tensor.wait_ge(s_dve, mark_dve_rel[jp])
                    bh, bw = rel_banks(j)
                    # M=128 table windows (f32r mm dst must start at
                    # partition 0): relh_j lands at rows 64:96 of bh,
                    # relw_j at rows 96:128 of bw; other rows are junk
                    tensor.matmul(
                        bh[0:128, 0:384],
                        relh[:, j * 32:j * 32 + 128],
                        qaug[0:64, :, j * 32:(j + 1) * 32],
                        start=True, stop=True,
                    ).then_inc(s_pe, 1)
                    tensor.matmul(
                        bw[0:128, 0:384],
                        relw[:, j * 32:j * 32 + 128],
                        qaug4[0:64, :, :, j],
                        start=True, stop=True,
                    ).then_inc(s_pe, 1)
                    C += 2
                    assert C == mark_pe_rel[j]
                elif ev == "k":
                    g = arg
                    jt, b = (g - 12) // 2, (g - 12) % 2
                    tensor.wait_ge(s_act, mark_act_grp[g - 4])
                    if first_k:
                        tensor.wait_ge(s_ld2, 16)
                        first_k = False
                    for dt in range(6):
                        mm = tensor.matmul(
                            banks4[g % 4],
                            wB[:, dt, jt * 128:(jt + 1) * 128],
                            xT[:, dt, b * 512:(b + 1) * 512],
                            start=(dt == 0), stop=(dt == 5),
                        )
                    mm.then_inc(s_pe, 1)
                    C += 1
                    assert C == mark_pe_k[g]
                else:  # v prefill group (even -> v_ps, odd -> bc_ps)
                    gv = arg
                    hp, st = gv // 8, gv % 8
                    if first_v:
                        tensor.wait_ge(s_lw, 16)
                        first_v = False
                    if gv <= 1:
                        tensor.wait_ge(s_dve, mark_dve_rel[31])
                    else:
                        tensor.wait_ge(s_vdve, gv - 1)
                    vb = v_ps if gv % 2 == 0 else bc_ps
                    for dt in range(6):
                        mm = tensor.matmul(
                            vb[:, 0:128],
                            xT[:, dt, st * 128:(st + 1) * 128],
                            wv_sb[:, dt, hp * 128:(hp + 1) * 128],
                            start=(dt == 0), stop=(dt == 5),
                        )
                    mm.then_inc(s_vpe, 1)
            assert C == PE0

            # --- phase 2 gates ---
            tensor.wait_ge(s_act, ACT0)
            tensor.wait_ge(s_dve, DVE0)
            tensor.wait_ge(s_qh, 6 * 16)
            tensor.wait_ge(s_kh[0], 12 * 16)
            tensor.wait_ge(s_kh[1], 12 * 16)
            tensor.wait_ge(s_oh, 10 * 16)
            tensor.wait_ge(s_oh2, 2 * 16)
            tensor.wait_ge(s_lg, 8 * 16)
            tensor.wait_ge(s_vdve, 24)

            # --- phase 2: attention iterations ---
            # PV4-7 of iter j are emitted after QK0/QK1 of iter j+1 so the
            # ACT exp stream never waits on tail PVs (gapless ACT).
            def pv(tt, fit):
                fm, fb = iters[fit]
                if tt % 2 == 0:
                    tensor.wait_ge(s_act, ACT0 + 4 * fit + (tt // 2) + 1)
                if fm >= 6:
                    tensor.wait_ge(s_vdve, (fm // 2) * 8 + tt + 1)
                if tt == 0 and fit >= 2:
                    tensor.wait_ge(s_dve, DVE0 + 4 * (fit - 2) + 4)
                base = (fm // 2) * 192 + (64 if fm % 2 else 0)
                tensor.matmul(
                    out_ps[fit % 2][0:128, :],
                    vaug[:, tt, base:base + 128],
                    exp_sb[:, fit % 2, tt * 512:(tt + 1) * 512],
                    start=(tt == 0), stop=(tt == 7),
                ).then_inc(s_pe, 1)

            for it, (m, b) in enumerate(iters):
                pb = pe_base(it)
                qrhs = qaug[:, m, b * 512:(b + 1) * 512]

                def qk(t):
                    if it >= 1:
                        if t == 0:
                            tensor.wait_ge(s_act, ACT0 + 4 * (it - 1) + 3)
                        elif t == 2:
                            tensor.wait_ge(s_act, ACT0 + 4 * (it - 1) + 4)
                    if t == 4:
                        tensor.wait_ge(s_act, ACT0 + 4 * it + 1)
                    elif t == 6:
                        tensor.wait_ge(s_act, ACT0 + 4 * it + 2)
                    tens = [0, 1, 0, 1][t // 2]   # pair p -> A B A B
                    half = t % 2
                    tensor.matmul(
                        qk2[tens][:, half * 512:(half + 1) * 512],
                        kaug[:, m, t * 128:(t + 1) * 128],
                        qrhs,
                        start=True, stop=True,
                    ).then_inc(s_pe, 1)

                def filler():
                    # harmless target: out_ps[1] is always rewritten by a
                    # start=True PV accumulation before any read
                    tensor.matmul(
                        out_ps[1][0:64, 0:64], warm[0:1, 0:64],
                        warm[0:1, 0:64],
                        start=True, stop=True,
                    ).then_inc(s_pe, 1)

                qk(0)                       # inc 1
                qk(1)                       # inc 2
                # deferred v groups of the previous iteration (moved off the
                # iteration boundary so pair0 never waits behind them);
                # even -> v_ps, odd -> bc_ps
                if it >= 1:
                    for gv in v_sched[it - 1]:
                        hp, st = 3 + (gv - 24) // 8, (gv - 24) % 8
                        tensor.wait_ge(s_vdve, gv - 1)
                        vb = v_ps if gv % 2 == 0 else bc_ps
                        for dt in range(6):
                            mm = tensor.matmul(
                                vb[:, 0:128],
                                xT[:, dt, st * 128:(st + 1) * 128],
                                wv_sb[:, dt, hp * 128:(hp + 1) * 128],
                                start=(dt == 0), stop=(dt == 5),
                            )
                        mm.then_inc(s_vpe, 1)
                if it >= 1:                  # incs 3,4: prev iter PV4,PV5
                    pv(4, it - 1)
                    pv(5, it - 1)
                else:
                    filler()
                    filler()
                qk(2)                       # inc 5
                qk(3)                       # inc 6
                for t in range(4, 8):        # incs 7..10
                    qk(t)
                if it >= 1:                  # incs 11,12: prev iter PV6,PV7
                    pv(6, it - 1)
                    pv(7, it - 1)
                else:
                    filler()
                    filler()
                for tt in (0, 1, 2, 3):      # incs 13..16
                    pv(tt, it)
                C += 16
                assert C == pb + 16

            # tail: PV4-7(23)
            for tt in (4, 5, 6, 7):
                pv(tt, N_ITERS - 1)          # PE0+384 + 1..4
            C += 4

            # --- phase 3: output projection ---
            PRJ0 = C
            tensor.wait_ge(s_dve, DVE0 + 4 * N_ITERS)
            tensor.wait_ge(s_ld3, 16)
            tensor.wait_ge(s_act, ACT0 + 96)
            for jt in range(6):
                T = qk2[jt % 2]
                if jt >= 2:
                    tensor.wait_ge(s_act, ACT0 + 96 + (jt - 2) + 1)
                for b in range(2):
                    for ct in range(6):
                        mm = tensor.matmul(
                            T[:, b * 512:(b + 1) * 512],
                            wproj[:, ct, jt * 128:(jt + 1) * 128],
                            outdT[:, ct, b * 512:(b + 1) * 512],
                            start=(ct == 0), stop=(ct == 5),
                        )
                    mm.then_inc(s_pe, 1)
                    C += 1

        block.tensor(_pe)

        # ================= scalar (ACT) =================
        def _act(scalar):
            # small const loads on the ACT hwdge ring
            scalar.dma_start(out=xT[:, 3:6, :], in_=xT_e[:, 3 * S:6 * S]
                             ).then_inc(s_ld1, 16)
            scalar.dma_start(out=wA[:, 3:6, :], in_=wqk_e[:, 3 * DIM:6 * DIM]
                             ).then_inc(s_ld1, 16)
            scalar.dma_start(out=qkb_sb[:], in_=qkb_e[:]).then_inc(s_lc, 16)
            scalar.dma_start(out=projb_sb[:], in_=projb_e[:]).then_inc(s_lc, 16)
            scalar.dma_start(out=wv_sb[:], in_=wv_e[:]).then_inc(s_lw, 16)
            scalar.wait_ge(s_ls, 3 * 16)
            for mm_ in range(10):
                scalar.dma_start(out=kaug[64:128, mm_, :], in_=oneh_sb[:],
                                 ).then_inc(s_oh, 16)
            # preload the exp table set off the critical path
            scalar.wait_ge(s_init, 1)
            scalar.activation(dummy[:], dummy[:], AF.Exp, scale=0.0)

            cA = 0
            # phase 1A: q copies (with bias; odd head staged in rows 64:128)
            scalar.wait_ge(s_lc, 2 * 16)
            for g in range(12):
                jt, b = g // 2, g % 2
                scalar.wait_ge(s_pe, g + 1)
                scalar.activation(
                    qaug[:, 2 * jt, b * 512:(b + 1) * 512],
                    banks4[g % 4],
                    AF.Identity,
                    bias=qkb_sb[:, jt:jt + 1],
                ).then_inc(s_act, 1)
                cA += 1
            # phase 1B: k copies + even rel pairs, in PE production order
            qaug4 = qaug[:].rearrange("p m (h w) -> p m h w", w=32)
            for ev, arg in events1b:
                if ev == "k":
                    g = arg
                    jt, b = (g - 12) // 2, (g - 12) % 2
                    scalar.wait_ge(s_pe, mark_pe_k[g])
                    if g - 12 >= 2:
                        # kscr slot WAR: both hops of group g-2 done
                        scalar.wait_ge(s_kh[g % 2], ((g - 12) // 2) * 32)
                    scalar.activation(
                        kscr[g % 2][:, :],
                        banks4[g % 4],
                        AF.Identity,
                        bias=qkb_sb[:, 6 + jt:6 + jt + 1],
                    ).then_inc(s_act, 1)
                    cA += 1
                    assert cA == mark_act_grp[g]
                elif ev == "rel" and arg % 2 == 0:
                    j = arg
                    bh, bw = rel_banks(j)
                    scalar.wait_ge(s_pe, mark_pe_rel[j])
                    scalar.activation(
                        qaug[64:96, :, j * 32:(j + 1) * 32],
                        bh[64:96, 0:384].rearrange("p (m w) -> p m w", w=32),
                        AF.Copy,
                    ).then_inc(s_act, 1)
                    scalar.activation(
                        qaug4[96:128, :, :, j],
                        bw[96:128, 0:384].rearrange("p (m h) -> p m h", h=32),
                        AF.Copy,
                    ).then_inc(s_act, 1)
                    cA += 2
                    assert cA == mark_act_rel[j]
            assert cA == ACT0

            # phase 2: exps only (4 per iter, [128,1024] across 2 banks)
            for it in range(N_ITERS):
                pb = pe_base(it)
                for p in range(4):
                    scalar.wait_ge(s_pe, pb + [2, 6, 8, 10][p])
                    if it >= 2:
                        if p < 2:   # read by PV0-3 of it-2 (in iter it-2)
                            scalar.wait_ge(
                                s_pe, pe_base(it - 2) + [14, 16][p])
                        else:       # read by PV4-7 of it-2 (in iter it-1)
                            scalar.wait_ge(
                                s_pe, pe_base(it - 1) + [4, 12][p - 2])
                    scalar.activation(
                        exp_sb[:, it % 2, p * 1024:(p + 1) * 1024],
                        qk2[p % 2][:, 0:1024],
                        AF.Exp,
                    ).then_inc(s_act, 1)
                    cA += 1
            assert cA == ACT0 + 96

            # phase 3: paired copies with bias
            prj0 = PE0 + 16 * N_ITERS + 4   # s_pe after the tail PVs
            for jt in range(6):
                scalar.wait_ge(s_pe, prj0 + 2 * jt + 2)
                if jt >= 2:
                    scalar.wait_ge([s_out0, s_out1][jt % 2], (jt // 2) * 16)
                scalar.activation(
                    out_sb[jt % 2][:, 0:1024],
                    qk2[jt % 2][:, 0:1024],
                    AF.Identity,
                    bias=projb_sb[:, jt:jt + 1],
                ).then_inc(s_act, 1)
                cA += 1

        block.scalar(_act)

        # ================= vector (DVE) =================
        def _dve(vector):
            vector.memset(krow[:, :], 0x7EF477D5)
            vector.memset(warm[0:1, :], 0.5)
            vector.memset(dummy[0:1, :], 0.0).then_inc(s_init, 1)
            va6 = vaug[:].rearrange("p t (hp blk) -> p t hp blk", blk=192)
            qaug4 = qaug[:].rearrange("p m (h w) -> p m h w", w=32)
            seed_f = seed_sb[:].bitcast(F32)
            cD = 0
            cV = 0
            # phase 1B: odd rel pairs + v prefill copies, in PE order
            for ev, arg in events1b:
                if ev == "rel" and arg % 2 == 1:
                    j = arg
                    bh, bw = rel_banks(j)
                    vector.wait_ge(s_pe, mark_pe_rel[j])
                    vector.tensor_copy(
                        qaug[64:96, :, j * 32:(j + 1) * 32],
                        bh[64:96, 0:384].rearrange("p (m w) -> p m w", w=32),
                    ).then_inc(s_dve, 1)
                    vector.tensor_copy(
                        qaug4[96:128, :, :, j],
                        bw[96:128, 0:384].rearrange("p (m h) -> p m h", h=32),
                    ).then_inc(s_dve, 1)
                    cD += 2
                    assert cD == mark_dve_rel[j]
                elif ev == "v":
                    gv = arg
                    hp, st = gv // 8, gv % 8
                    vector.wait_ge(s_vpe, gv + 1)
                    src = (v_ps if gv % 2 == 0 else bc_ps)[:, 0:128]
                    vector.tensor_copy(
                        va6[:, st, hp, :].rearrange(
                            "p (a c) -> p a c", c=64)[:, 0:3:2, :],
                        src.rearrange("p (m c) -> p m c", c=64),
                    ).then_inc(s_vdve, 1)
                    cV += 1
            assert cD == DVE0 and cV == 24

            # phase 2 per iter: recip chain of it, v copies of it+1 (they
            # must precede bccopy(it): bcast(it) at iter it+2 waits them),
            # then bccopy/mul of it
            def vcopies(vit):
                nonlocal cV
                for gv in v_sched[vit]:
                    vector.wait_ge(s_vpe, gv + 1)
                    hp, st = 3 + (gv - 24) // 8, (gv - 24) % 8
                    vsrc = (v_ps if gv % 2 == 0 else bc_ps)[:, 0:128]
                    vector.tensor_copy(
                        va6[:, st, hp, :].rearrange(
                            "p (a c) -> p a c", c=64)[:, 0:3:2, :],
                        vsrc.rearrange("p (m c) -> p m c", c=64),
                    ).then_inc(s_vdve, 1)
                    cV += 1

            vcopies(0)
            for it, (m, b) in enumerate(iters):
                pb = pe_base(it)
                # denominator rows sit at the opposite half from the v rows
                # (head-parity); the chain runs at the v half reading the
                # denominator half through the PSUM operand (walrus allows
                # cross-base only for PSUM APs). magic-seed + 1 NR.
                dh = 64 if m % 2 == 0 else 0
                vh = 64 - dh
                sv = slice(vh, vh + 64)
                drow = out_ps[it % 2][dh:dh + 64, :]
                vector.wait_ge(s_pe, pv7_inc(it))
                vector.tensor_sub(
                    seed_sb[sv, :], krow[sv, :], drow.bitcast(I32),
                ).then_inc(s_dve, 1)
                vector.scalar_tensor_tensor(
                    t_sb[sv, :], drow, -1.0, seed_f[sv, :],
                    op0=ALU.mult, op1=ALU.mult,
                ).then_inc(s_dve, 1)
                vector.scalar_tensor_tensor(
                    recip_sb[sv, :], t_sb[sv, :], 2.0, seed_f[sv, :],
                    op0=ALU.add, op1=ALU.mult,
                ).then_inc(s_dve, 1)
                vector.tensor_mul(
                    outdT[sv, m // 2, b * 512:(b + 1) * 512],
                    out_ps[it % 2][sv, :],
                    recip_sb[sv, :],
                ).then_inc(s_dve, 1)
                cD += 4
                assert cD == DVE0 + 4 * (it + 1)
                if it + 1 < N_ITERS:
                    vcopies(it + 1)

        block.vector(_dve)

    # clear semaphores so the NEFF is safely re-executable
    nc.reset()
    return nc


def _prep_inputs(x, qkv_w, qkv_b, proj_w, proj_b, rel_pos_h, rel_pos_w):
    """Host-side constant prep shared across cores (everything but xT)."""
    import ml_dtypes
    bf = ml_dtypes.bfloat16
    f32 = np.float32
    wq = qkv_w[0:DIM].astype(f32) * SCALE
    wk = qkv_w[DIM:2 * DIM].astype(f32)
    wv = qkv_w[2 * DIM:3 * DIM].astype(f32)
    def shuf(a):
        # [768, N] -> [128, 6*N]: row p holds all 6 dt-tiles for partition p
        n = a.shape[1]
        return np.ascontiguousarray(
            a.reshape(6, 128, n).transpose(1, 0, 2).reshape(128, 6 * n))

    wqk = np.concatenate(
        [shuf(wq.T), shuf(wk.T)], axis=1).astype(bf).copy()
    wv_t = shuf(wv.T).astype(bf).copy()
    wproj = shuf(proj_w.astype(f32).T).astype(bf).copy()

    qb = qkv_b[0:DIM].astype(f32) * SCALE
    kb = qkv_b[DIM:2 * DIM].astype(f32)
    vb = qkv_b[2 * DIM:3 * DIM].astype(f32)
    qkb = np.concatenate(
        [qb.reshape(6, 128).T, kb.reshape(6, 128).T], axis=1).copy()
    projb_eff = (proj_b.astype(f32) + vb @ proj_w.astype(f32).T)
    projb = projb_eff.reshape(6, 128).T.copy()

    idx = np.arange(H)[:, None] - np.arange(H)[None, :] + (H - 1)
    Rh = rel_pos_h.astype(f32)[idx]
    Rw = rel_pos_w.astype(f32)[idx]
    relh_core = (Rh.transpose(2, 0, 1) / SCALE).reshape(HD, H * H)
    relw_core = (Rw.transpose(2, 0, 1) / SCALE).reshape(HD, W * W)
    # M=64 window tables: relh_j at block j (one pad block after);
    # relw_j at block j+1 (one pad block before)
    relh = np.zeros((HD, 35 * 32), dtype=f32)
    relh[:, 64:64 + 1024] = relh_core    # relh_j at window cols 64:96
    relw = np.zeros((HD, 35 * 32), dtype=f32)
    relw[:, 96:96 + 1024] = relw_core    # relw_j at window cols 96:128

    onehot = np.zeros((HD, S), dtype=f32)
    s = np.arange(S)
    onehot[s // W, s] = 1.0
    onehot[32 + s % W, s] = 1.0
    onescol = np.ones((128, 6 * 64), dtype=bf)

    return dict(wqk=wqk, wv=wv_t, wproj=wproj, relh=relh, relw=relw,
                onehot=onehot, onescol=onescol,
                qkb=qkb, projb=projb)


_CACHED_NC = None


def kernel(x, qkv_w, qkv_b, proj_w, proj_b, rel_pos_h, rel_pos_w,
           trace=False):
    import ml_dtypes
    from concourse.bass_utils import run_bass_kernel_spmd

    global _CACHED_NC
    if _CACHED_NC is None:
        _CACHED_NC = build_nc()
    nc = _CACHED_NC

    consts = _prep_inputs(x, qkv_w, qkv_b, proj_w, proj_b,
                          rel_pos_h, rel_pos_w)
    bf = ml_dtypes.bfloat16
    in_maps = []
    for bb in range(NCORES):
        xTf = np.asarray(x[bb]).reshape(S, DIM).T.reshape(6, 128, S)
        xT = np.ascontiguousarray(
            xTf.transpose(1, 0, 2).reshape(128, 6 * S)).astype(bf)
        in_maps.append({"xT": xT, **consts})

    res = run_bass_kernel_spmd(nc, in_maps, core_ids=list(range(NCORES)),
                               trace=trace)
    outs = []
    for bb in range(NCORES):
        outT = np.asarray(res.results[bb]["outT"]).astype(np.float32)
        outs.append(outT.T.reshape(H, W, DIM))
    full = np.stack(outs, axis=0).astype(np.float32)
    if trace:
        return full, res
    return full



# revision 3
# speedup vs baseline: 1.0045x; 1.0045x over previous
"""Trainium2 Bass kernel for MViTv2-style attention (decomposed rel-pos bias).

Problem: B=8, H=W=32, DIM=768, NH=12, HD=64, S=1024.
Sharding: data-parallel, one batch element per NeuronCore (8 cores).

v2 redesign vs baseline (450us):
  - all DRAM tensors bf16 (16.3MB -> ~6.6MB HBM), matmuls in bf16
  - loads interleaved with q-projection matmuls (per-dt gating)
  - one ACT bias-copy per q/k group ([128,512]); odd head staged in rows
    64:128 of the even head's qaug slot, then ONE sbuf-sbuf hop DMA
  - rel-pos matmuls land on partitions 64:96 / 96:128 directly via
    col-tiled tile_position -> no partition-shift DMAs, plain copies
  - phase 2: exp reads [128,1024] across 2 psum banks (4 ACT ops/iter);
    softmax denominator chain fully on DVE (reciprocal_approx_fast);
    v-projection matmuls deferred into phase 2 to keep PE busy (HAM)
  - phase 3: paired [128,1024] ACT copies, bf16 output
"""
import numpy as np

B, H, W, DIM, NH = 8, 32, 32, 768, 12
HD = DIM // NH          # 64
S = H * W               # 1024
SCALE = HD ** -0.5
NCORES = 8


N_ITERS = 24


V_PREFILL = 15


def _v_sched():
    """v-group ids per phase-2 iteration: 33 deferred groups, all
    scheduled by iter 20 (deadline: iter 4*hp for group (hp, st))."""
    per_iter = [2] * 13 + [1] * 7 + [0] * 4
    sched, gv = [], V_PREFILL
    for n in per_iter:
        sched.append(list(range(gv, gv + n)))
        gv += n
    assert gv == 48
    return sched


def _phase1b_events():
    """PE-order event plan for phase 1B: rel pairs + k groups, then v."""
    events = []
    for j in range(32):
        events.append(("rel", j))
        if j % 2 == 1 and (j - 1) // 2 < 12:
            events.append(("k", 12 + (j - 1) // 2))
    for gv in range(V_PREFILL):
        events.append(("v", gv))
    return events


def build_nc():
    import concourse.bass as bass
    import concourse.mybir as mybir
    from contextlib import ExitStack

    F32 = mybir.dt.float32
    F32R = mybir.dt.float32r
    BF16 = mybir.dt.bfloat16
    I32 = mybir.dt.int32
    AF = mybir.ActivationFunctionType
    ALU = mybir.AluOpType

    nc = bass.Bass(detect_race_conditions=False)

    # ---- DRAM parameters (per core) ----
    # host pre-shuffles (dt p) -> p-major so each partition is one
    # contiguous DRAM run (128 big DMA descriptors instead of 768 small)
    xT_e = nc.declare_dram_parameter("xT", [128, 6 * S], BF16, isOutput=False)
    wqk_e = nc.declare_dram_parameter("wqk", [128, 2 * 6 * DIM], BF16, isOutput=False)
    wv_e = nc.declare_dram_parameter("wv", [128, 6 * DIM], BF16, isOutput=False)
    wproj_e = nc.declare_dram_parameter("wproj", [128, 6 * DIM], BF16, isOutput=False)
    relh_e = nc.declare_dram_parameter("relh", [HD, 35 * 32], F32R, isOutput=False)
    relw_e = nc.declare_dram_parameter("relw", [HD, 35 * 32], F32R, isOutput=False)
    oneh_e = nc.declare_dram_parameter("onehot", [HD, S], F32R, isOutput=False)
    onescol_e = nc.declare_dram_parameter("onescol", [128, 6 * 64], BF16, isOutput=False)
    qkb_e = nc.declare_dram_parameter("qkb", [128, 12], F32, isOutput=False)
    projb_e = nc.declare_dram_parameter("projb", [128, 6], F32, isOutput=False)
    outT_e = nc.declare_dram_parameter("outT", [DIM, S], BF16, isOutput=True)

    events1b = _phase1b_events()
    v_sched = _v_sched()

    ctx = ExitStack()
    with ctx:
        # ---- SBUF ----
        # f32r for qaug/kaug: bf16 strided writes run ~3.3x slow on ACT/DVE
        # (the rel-w copies scatter into qaug with stride 32) and walrus
        # rejects mixed 32/16-bit matmul inputs, so both aug tensors are
        # 4-byte. To fit 192KB, phase-1-only tensors (wA/wB/oneh/kscr/rel
        # tables) share their region with tensors first written in phase 2+
        # (exp_sb/ttscr/seed/t/recip/bcast/out_sb) via stack scoping; the
        # phase-2 gates guarantee all reuse is ordered at runtime.
        xT = ctx.enter_context(nc.sbuf_tensor("xT_sb", [128, 6, S], BF16))
        wv_sb = ctx.enter_context(nc.sbuf_tensor("wv_sb", [128, 6, DIM], BF16))
        wproj = ctx.enter_context(nc.sbuf_tensor("wproj_sb", [128, 6, DIM], BF16))
        qaug = ctx.enter_context(nc.sbuf_tensor("qaug", [128, NH, S], F32R))
        kaug = ctx.enter_context(nc.sbuf_tensor("kaug", [128, NH, S], F32R))
        # vaug per tt: [v0|ones|v1  v2|ones|v3 ...] (6 pairs x 192); head m
        # lhsT = cols (m//2)*192 + (64 if m odd else 0), width 128; the ones
        # block doubles as the softmax-denominator column bundle, so PV
        # output rows split into v-rows and 64 denominator rows by parity
        vaug = ctx.enter_context(nc.sbuf_tensor("vaug", [128, 8, 1152], BF16))
        outdT = ctx.enter_context(nc.sbuf_tensor("outdT_sb", [128, 6, S], BF16))
        krow = ctx.enter_context(nc.sbuf_tensor("krow", [128, 512], I32))
        qkb_sb = ctx.enter_context(nc.sbuf_tensor("qkb_sb", [128, 12], F32))
        projb_sb = ctx.enter_context(nc.sbuf_tensor("projb_sb", [128, 6], F32))
        dummy = ctx.enter_context(nc.sbuf_tensor("expwarm", [1, 8], F32))
        warm = ctx.enter_context(nc.sbuf_tensor("warm", [1, 512], BF16))
        # phase-1-only scope (freed below; region reused by phase-2 tensors)
        p1 = ExitStack()
        wA = p1.enter_context(nc.sbuf_tensor("wA", [128, 6, DIM], BF16))
        wB = p1.enter_context(nc.sbuf_tensor("wB", [128, 6, DIM], BF16))
        oneh_sb = p1.enter_context(nc.sbuf_tensor("oneh_sb", [64, S], F32R))
        relh = p1.enter_context(nc.sbuf_tensor("relh_sb", [HD, 35 * 32], F32R))
        relw = p1.enter_context(nc.sbuf_tensor("relw_sb", [HD, 35 * 32], F32R))
        kscr = [p1.enter_context(
            nc.sbuf_tensor(f"kscr{o}", [128, 512], F32R)) for o in range(2)]
        p1.close()
        # phase-2+ tensors, overlapping the freed phase-1 region
        exp_sb = ctx.enter_context(nc.sbuf_tensor("exp_sb", [128, 2, 8 * 512], BF16))
        seed_sb = ctx.enter_context(nc.sbuf_tensor("seed_sb", [128, 512], I32))
        t_sb = ctx.enter_context(nc.sbuf_tensor("t_sb", [128, 512], F32))
        recip_sb = ctx.enter_context(nc.sbuf_tensor("recip_sb", [128, 512], F32))
        out_sb = [ctx.enter_context(
            nc.sbuf_tensor(f"out_sb{o}", [128, 1024], BF16)) for o in range(2)]

        # ---- PSUM (8 banks) ----
        qkA = ctx.enter_context(nc.psum_tensor("qkA", [128, 1024], F32))
        qkB = ctx.enter_context(nc.psum_tensor("qkB", [128, 1024], F32))
        out_ps = [ctx.enter_context(nc.psum_tensor(f"out_ps{o}", [128, 512], F32))
                  for o in range(2)]
        # bc_ps rows 0:64 hold the bcast result; odd v-groups borrow the bank
        bc_ps = ctx.enter_context(nc.psum_tensor("bc_ps", [128, 512], F32))
        v_ps = ctx.enter_context(nc.psum_tensor("v_ps", [128, 512], F32))

        banks4 = [qkA[:, 0:512], qkA[:, 512:1024],
                  qkB[:, 0:512], qkB[:, 512:1024]]
        qk2 = [qkA, qkB]

        # ---- semaphores ----
        s_ld1 = ctx.enter_context(nc.semaphore("s_ld1"))  # xT + wA loads
        s_ld2 = ctx.enter_context(nc.semaphore("s_ld2"))  # wB loads
        s_ld3 = ctx.enter_context(nc.semaphore("s_ld3"))  # wproj loads
        s_ls = ctx.enter_context(nc.semaphore("s_ls"))    # rel tables (gpsimd)
        s_lc = ctx.enter_context(nc.semaphore("s_lc"))    # tiny consts (scalar)
        s_lw = ctx.enter_context(nc.semaphore("s_lw"))    # wv loads (scalar q)
        s_lg = ctx.enter_context(nc.semaphore("s_lg"))    # onescol (gpsimd)
        s_qh = ctx.enter_context(nc.semaphore("s_qh"))    # q odd-head hops
        s_kh = [ctx.enter_context(nc.semaphore(f"s_kh{o}"))
                for o in range(2)]                        # k hops per kscr slot
        s_oh = ctx.enter_context(nc.semaphore("s_oh"))    # onehot repl (ACT ring)
        s_oh2 = ctx.enter_context(nc.semaphore("s_oh2"))  # onehot repl (gpsimd)
        s_out0 = ctx.enter_context(nc.semaphore("s_out0"))
        s_out1 = ctx.enter_context(nc.semaphore("s_out1"))
        s_pe = ctx.enter_context(nc.semaphore("s_pe"))
        s_act = ctx.enter_context(nc.semaphore("s_act"))
        s_dve = ctx.enter_context(nc.semaphore("s_dve"))
        s_init = ctx.enter_context(nc.semaphore("s_init"))  # DVE memset init
        s_vpe = ctx.enter_context(nc.semaphore("s_vpe"))    # v matmul groups
        s_vdve = ctx.enter_context(nc.semaphore("s_vdve"))  # v psum->vaug copies

        block = ctx.enter_context(nc.Block())

        # ---- analytic semaphore layout ----
        # phase 1: s_pe: q groups 1..12, then events1b: k groups / rel mms
        # (2 per pair); s_act: q copies 1..12, then per events1b (k copy 1,
        # rel-even pair 2); s_dve: rel-odd pairs (2 each) = 32 total
        pe_n = 12
        act_n = 12
        dve_n = 0
        mark_pe_k = {}        # k group -> s_pe after its stop mm
        mark_pe_rel = {}      # rel pair -> s_pe after both mms
        mark_act_grp = {}     # q/k group -> s_act after its copy
        for g in range(12):
            mark_act_grp[g] = g + 1
        mark_act_rel = {}     # even rel pair -> s_act after 2 copies
        mark_dve_rel = {}     # odd rel pair -> s_dve after 2 copies
        for ev, arg in events1b:
            if ev == "rel":
                pe_n += 2
                mark_pe_rel[arg] = pe_n
                if arg % 2 == 0:
                    act_n += 2
                    mark_act_rel[arg] = act_n
                else:
                    dve_n += 2
                    mark_dve_rel[arg] = dve_n
            elif ev == "k":
                pe_n += 1
                mark_pe_k[arg] = pe_n
                act_n += 1
                mark_act_grp[arg] = act_n
        PE0, ACT0, DVE0 = pe_n, act_n, dve_n    # 88, 56, 32
        assert (PE0, ACT0, DVE0) == (88, 56, 32)

        def rel_banks(j):
            # (h-mm bank, w-mm bank) for rel pair j; two bank-sets, depth-2
            return ([out_ps[0], out_ps[1]], [bc_ps, v_ps])[j % 2]

        def rel_prev(j):
            # previous rel pair that used the same bank set
            return j - 2 if j >= 2 else None

        def pe_base(it):
            return PE0 + 16 * it

        def pv7_inc(j):
            return pe_base(j + 1) + 12 if j < N_ITERS - 1 else PE0 + 16 * N_ITERS + 4

        iters = [(m, b) for m in range(NH) for b in range(2)]

        # ================= sync queue: loads, hops, out =================
        def _sync(sync):
            sync.dma_start(out=xT[:, 0:3, :], in_=xT_e[:, 0:3 * S]
                           ).then_inc(s_ld1, 16)
            sync.dma_start(out=wA[:, 0:3, :], in_=wqk_e[:, 0:3 * DIM]
                           ).then_inc(s_ld1, 16)
            sync.dma_start(out=wB[:], in_=wqk_e[:, 6 * DIM:12 * DIM]
                           ).then_inc(s_ld2, 16)
            # q odd-head hops: qaug[64:128, 2jt] -> qaug[0:64, 2jt+1],
            # one full-row DMA per head pair (waits both halves' copies)
            for jt in range(6):
                sync.wait_ge(s_act, mark_act_grp[2 * jt + 1])
                sync.dma_start(
                    out=qaug[0:64, 2 * jt + 1, :],
                    in_=qaug[64:128, 2 * jt, :],
                ).then_inc(s_qh, 16)
            # k hops: kscr[g%2] rows 0:64 -> kaug[0:64, 2jt] (even head),
            # rows 64:128 -> kaug[0:64, 2jt+1] (odd head)
            for g in range(12, 24):
                jt, b = (g - 12) // 2, (g - 12) % 2
                sync.wait_ge(s_act, mark_act_grp[g])
                sync.dma_start(
                    out=kaug[0:64, 2 * jt, b * 512:(b + 1) * 512],
                    in_=kscr[g % 2][0:64, :],
                ).then_inc(s_kh[g % 2], 16)
                sync.dma_start(
                    out=kaug[0:64, 2 * jt + 1, b * 512:(b + 1) * 512],
                    in_=kscr[g % 2][64:128, :],
                ).then_inc(s_kh[g % 2], 16)
            sync.dma_start(out=wproj[:], in_=wproj_e[:]).then_inc(s_ld3, 16)
            # phase-3 output stores
            for jt in range(6):
                sync.wait_ge(s_act, ACT0 + 96 + jt + 1)
                sync.dma_start(
                    out=outT_e[jt * 128:(jt + 1) * 128, :],
                    in_=out_sb[jt % 2][:, 0:1024],
                ).then_inc([s_out0, s_out1][jt % 2], 16)
            sync.wait_ge(s_out0, 3 * 16)
            sync.wait_ge(s_out1, 3 * 16)

        block.sync(_sync)

        # ================= gpsimd queue: onescol + onehot repl =================
        def _gp(gp):
            gp.dma_start(out=relh[:], in_=relh_e[:]).then_inc(s_ls, 16)
            gp.dma_start(out=relw[:], in_=relw_e[:]).then_inc(s_ls, 16)
            gp.dma_start(out=oneh_sb[:], in_=oneh_e[:]).then_inc(s_ls, 16)
            va6 = vaug[:].rearrange("p t (hp blk) -> p t hp blk", blk=192)
            with nc.allow_non_contiguous_dma(reason="64-col ones blocks"):
                for sk in range(8):
                    gp.dma_start(out=va6[:, sk, :, 64:128],
                                 in_=onescol_e[:].rearrange(
                                     "p (hp c) -> p hp c", c=64),
                                 ).then_inc(s_lg, 16)
            # onehot rows into kaug[64:128, m] for heads 6..11 (0..5 go
            # via the scalar ring; kaug upper rows have no other writers)
            # FIFO after the oneh load on this same ring -- no wait needed
            for mm_ in range(10, NH):
                gp.dma_start(out=kaug[64:128, mm_, :], in_=oneh_sb[:],
                             ).then_inc(s_oh2, 16)

        block.gpsimd(_gp)

        # ================= PE =================
        def _pe(tensor):
            C = 0
            # warm up the PE clock (HAM ramps ~3.4us) while loads stream
            tensor.wait_ge(s_init, 1)
            for _ in range(10):
                tensor.matmul(
                    bc_ps[0:64, :], warm[0:1, 0:64], warm[0:1, :],
                    start=True, stop=True,
                )
            # --- phase 1A: q then (in events) k projections ---
            for g in range(12):
                jt, b = g // 2, g % 2
                if g >= 4:
                    tensor.wait_ge(s_act, mark_act_grp[g - 4])
                if g == 0:
                    tensor.wait_ge(s_ld1, 4 * 16)
                for dt in range(6):
                    mm = tensor.matmul(
                        banks4[g % 4],
                        wA[:, dt, jt * 128:(jt + 1) * 128],
                        xT[:, dt, b * 512:(b + 1) * 512],
                        start=(dt == 0), stop=(dt == 5),
                    )
                mm.then_inc(s_pe, 1)
                C += 1
                assert C == g + 1
            # --- phase 1B: rel pairs + k groups + v prefill (planned) ---
            qaug4 = qaug[:].rearrange("p m (h w) -> p m h w", w=32)
            first_rel = True
            first_k = True
            first_v = True
            for ev, arg in events1b:
                if ev == "rel":
                    j = arg
                    if first_rel:
                        # keep the HAM clock ramped while the hop DMAs and
                        # table loads land (PE would otherwise idle ~8-12us
                        # and fall back to the 1.2GHz pstate for the rel mms)
                        for _ in range(20):
                            tensor.matmul(
                                bc_ps[0:64, :], warm[0:1, 0:64],
                                warm[0:1, :], start=True, stop=True,
                            )
                        tensor.wait_ge(s_qh, 6 * 16)   # all q hops
                        tensor.wait_ge(s_ls, 2 * 16)    # relh/relw loaded
                        first_rel = False
                    jp = rel_prev(j)
                    if jp is not None:
                        if jp % 2 == 0:
                            tensor.wait_ge(s_act, mark_act_rel[jp])
                        else:
                            tensor.wait_ge(s_dve, mark_dve_rel[jp])
                    bh, bw = rel_banks(j)
                    # M=128 table windows (f32r mm dst must start at
                    # partition 0): relh_j lands at rows 64:96 of bh,
                    # relw_j at rows 96:128 of bw; other rows are junk
                    tensor.matmul(
                        bh[0:128, 0:384],
                        relh[:, j * 32:j * 32 + 128],
                        qaug[0:64, :, j * 32:(j + 1) * 32],
                        start=True, stop=True,
                    ).then_inc(s_pe, 1)
                    tensor.matmul(
                        bw[0:128, 0:384],
                        relw[:, j * 32:j * 32 + 128],
                        qaug4[0:64, :, :, j],
                        start=True, stop=True,
                    ).then_inc(s_pe, 1)
                    C += 2
                    assert C == mark_pe_rel[j]
                elif ev == "k":
                    g = arg
                    jt, b = (g - 12) // 2, (g - 12) % 2
                    tensor.wait_ge(s_act, mark_act_grp[g - 4])
                    if first_k:
                        tensor.wait_ge(s_ld2, 16)
                        first_k = False
                    for dt in range(6):
                        mm = tensor.matmul(
                            banks4[g % 4],
                            wB[:, dt, jt * 128:(jt + 1) * 128],
                            xT[:, dt, b * 512:(b + 1) * 512],
                            start=(dt == 0), stop=(dt == 5),
                        )
                    mm.then_inc(s_pe, 1)
                    C += 1
                    assert C == mark_pe_k[g]
                else:  # v prefill group (even -> v_ps, odd -> bc_ps)
                    gv = arg
                    hp, st = gv // 8, gv % 8
                    if first_v:
                        tensor.wait_ge(s_lw, 16)
                        first_v = False
                    if gv <= 1:
                        tensor.wait_ge(s_dve, mark_dve_rel[31])
                    else:
                        tensor.wait_ge(s_vdve, gv - 1)
                    vb = v_ps if gv % 2 == 0 else bc_ps
                    for dt in range(6):
                        mm = tensor.matmul(
                            vb[:, 0:128],
                            xT[:, dt, st * 128:(st + 1) * 128],
                            wv_sb[:, dt, hp * 128:(hp + 1) * 128],
                            start=(dt == 0), stop=(dt == 5),
                        )
                    mm.then_inc(s_vpe, 1)
            assert C == PE0

            # --- phase 2 gates ---
            tensor.wait_ge(s_act, ACT0)
            tensor.wait_ge(s_dve, DVE0)
            tensor.wait_ge(s_qh, 6 * 16)
            tensor.wait_ge(s_kh[0], 12 * 16)
            tensor.wait_ge(s_kh[1], 12 * 16)
            tensor.wait_ge(s_oh, 10 * 16)
            tensor.wait_ge(s_oh2, 2 * 16)
            tensor.wait_ge(s_lg, 8 * 16)
            tensor.wait_ge(s_vdve, V_PREFILL)

            # --- phase 2: attention iterations ---
            # PV4-7 of iter j are emitted after QK0/QK1 of iter j+1 so the
            # ACT exp stream never waits on tail PVs (gapless ACT).
            def pv(tt, fit):
                fm, fb = iters[fit]
                if tt % 2 == 0:
                    tensor.wait_ge(s_act, ACT0 + 4 * fit + (tt // 2) + 1)
                if (fm // 2) * 8 + tt >= V_PREFILL:
                    tensor.wait_ge(s_vdve, (fm // 2) * 8 + tt + 1)
                if tt == 0 and fit >= 2:
                    tensor.wait_ge(s_dve, DVE0 + 4 * (fit - 2) + 4)
                base = (fm // 2) * 192 + (64 if fm % 2 else 0)
                tensor.matmul(
                    out_ps[fit % 2][0:128, :],
                    vaug[:, tt, base:base + 128],
                    exp_sb[:, fit % 2, tt * 512:(tt + 1) * 512],
                    start=(tt == 0), stop=(tt == 7),
                ).then_inc(s_pe, 1)

            for it, (m, b) in enumerate(iters):
                pb = pe_base(it)
                qrhs = qaug[:, m, b * 512:(b + 1) * 512]

                def qk(t):
                    if it >= 1:
                        if t == 0:
                            tensor.wait_ge(s_act, ACT0 + 4 * (it - 1) + 3)
                        elif t == 2:
                            tensor.wait_ge(s_act, ACT0 + 4 * (it - 1) + 4)
                    if t == 4:
                        tensor.wait_ge(s_act, ACT0 + 4 * it + 1)
                    elif t == 6:
                        tensor.wait_ge(s_act, ACT0 + 4 * it + 2)
                    tens = [0, 1, 0, 1][t // 2]   # pair p -> A B A B
                    half = t % 2
                    tensor.matmul(
                        qk2[tens][:, half * 512:(half + 1) * 512],
                        kaug[:, m, t * 128:(t + 1) * 128],
                        qrhs,
                        start=True, stop=True,
                    ).then_inc(s_pe, 1)

                def filler():
                    # harmless target: out_ps[1] is always rewritten by a
                    # start=True PV accumulation before any read
                    tensor.matmul(
                        out_ps[1][0:64, 0:64], warm[0:1, 0:64],
                        warm[0:1, 0:64],
                        start=True, stop=True,
                    ).then_inc(s_pe, 1)

                qk(0)                       # inc 1
                qk(1)                       # inc 2
                # deferred v groups of the previous iteration (moved off the
                # iteration boundary so pair0 never waits behind them);
                # even -> v_ps, odd -> bc_ps
                if it >= 1:
                    for gv in v_sched[it - 1]:
                        hp, st = gv // 8, gv % 8
                        tensor.wait_ge(s_vdve, gv - 1)
                        vb = v_ps if gv % 2 == 0 else bc_ps
                        for dt in range(6):
                            mm = tensor.matmul(
                                vb[:, 0:128],
                                xT[:, dt, st * 128:(st + 1) * 128],
                                wv_sb[:, dt, hp * 128:(hp + 1) * 128],
                                start=(dt == 0), stop=(dt == 5),
                            )
                        mm.then_inc(s_vpe, 1)
                if it >= 1:                  # incs 3,4: prev iter PV4,PV5
                    pv(4, it - 1)
                    pv(5, it - 1)
                else:
                    filler()
                    filler()
                qk(2)                       # inc 5
                qk(3)                       # inc 6
                for t in range(4, 8):        # incs 7..10
                    qk(t)
                if it >= 1:                  # incs 11,12: prev iter PV6,PV7
                    pv(6, it - 1)
                    pv(7, it - 1)
                else:
                    filler()
                    filler()
                for tt in (0, 1, 2, 3):      # incs 13..16
                    pv(tt, it)
                C += 16
                assert C == pb + 16

            # tail: PV4-7(23)
            for tt in (4, 5, 6, 7):
                pv(tt, N_ITERS - 1)          # PE0+384 + 1..4
            C += 4

            # --- phase 3: output projection ---
            PRJ0 = C
            tensor.wait_ge(s_dve, DVE0 + 4 * N_ITERS)
            tensor.wait_ge(s_ld3, 16)
            tensor.wait_ge(s_act, ACT0 + 96)
            for jt in range(6):
                T = qk2[jt % 2]
                if jt >= 2:
                    tensor.wait_ge(s_act, ACT0 + 96 + (jt - 2) + 1)
                for b in range(2):
                    for ct in range(6):
                        mm = tensor.matmul(
                            T[:, b * 512:(b + 1) * 512],
                            wproj[:, ct, jt * 128:(jt + 1) * 128],
                            outdT[:, ct, b * 512:(b + 1) * 512],
                            start=(ct == 0), stop=(ct == 5),
                        )
                    mm.then_inc(s_pe, 1)
                    C += 1

        block.tensor(_pe)

        # ================= scalar (ACT) =================
        def _act(scalar):
            # small const loads on the ACT hwdge ring
            scalar.dma_start(out=xT[:, 3:6, :], in_=xT_e[:, 3 * S:6 * S]
                             ).then_inc(s_ld1, 16)
            scalar.dma_start(out=wA[:, 3:6, :], in_=wqk_e[:, 3 * DIM:6 * DIM]
                             ).then_inc(s_ld1, 16)
            scalar.dma_start(out=qkb_sb[:], in_=qkb_e[:]).then_inc(s_lc, 16)
            scalar.dma_start(out=projb_sb[:], in_=projb_e[:]).then_inc(s_lc, 16)
            scalar.dma_start(out=wv_sb[:], in_=wv_e[:]).then_inc(s_lw, 16)
            scalar.wait_ge(s_ls, 3 * 16)
            for mm_ in range(10):
                scalar.dma_start(out=kaug[64:128, mm_, :], in_=oneh_sb[:],
                                 ).then_inc(s_oh, 16)
            # preload the exp table set off the critical path
            scalar.wait_ge(s_init, 1)
            scalar.activation(dummy[:], dummy[:], AF.Exp, scale=0.0)

            cA = 0
            # phase 1A: q copies (with bias; odd head staged in rows 64:128)
            scalar.wait_ge(s_lc, 2 * 16)
            for g in range(12):
                jt, b = g // 2, g % 2
                scalar.wait_ge(s_pe, g + 1)
                scalar.activation(
                    qaug[:, 2 * jt, b * 512:(b + 1) * 512],
                    banks4[g % 4],
                    AF.Identity,
                    bias=qkb_sb[:, jt:jt + 1],
                ).then_inc(s_act, 1)
                cA += 1
            # phase 1B: k copies + even rel pairs, in PE production order
            qaug4 = qaug[:].rearrange("p m (h w) -> p m h w", w=32)
            for ev, arg in events1b:
                if ev == "k":
                    g = arg
                    jt, b = (g - 12) // 2, (g - 12) % 2
                    scalar.wait_ge(s_pe, mark_pe_k[g])
                    if g - 12 >= 2:
                        # kscr slot WAR: both hops of group g-2 done
                        scalar.wait_ge(s_kh[g % 2], ((g - 12) // 2) * 32)
                    scalar.activation(
                        kscr[g % 2][:, :],
                        banks4[g % 4],
                        AF.Identity,
                        bias=qkb_sb[:, 6 + jt:6 + jt + 1],
                    ).then_inc(s_act, 1)
                    cA += 1
                    assert cA == mark_act_grp[g]
                elif ev == "rel" and arg % 2 == 0:
                    j = arg
                    bh, bw = rel_banks(j)
                    scalar.wait_ge(s_pe, mark_pe_rel[j])
                    scalar.activation(
                        qaug[64:96, :, j * 32:(j + 1) * 32],
                        bh[64:96, 0:384].rearrange("p (m w) -> p m w", w=32),
                        AF.Copy,
                    ).then_inc(s_act, 1)
                    scalar.activation(
                        qaug4[96:128, :, :, j],
                        bw[96:128, 0:384].rearrange("p (m h) -> p m h", h=32),
                        AF.Copy,
                    ).then_inc(s_act, 1)
                    cA += 2
                    assert cA == mark_act_rel[j]
            assert cA == ACT0

            # phase 2: exps only (4 per iter, [128,1024] across 2 banks)
            for it in range(N_ITERS):
                pb = pe_base(it)
                for p in range(4):
                    scalar.wait_ge(s_pe, pb + [2, 6, 8, 10][p])
                    if it >= 2:
                        if p < 2:   # read by PV0-3 of it-2 (in iter it-2)
                            scalar.wait_ge(
                                s_pe, pe_base(it - 2) + [14, 16][p])
                        else:       # read by PV4-7 of it-2 (in iter it-1)
                            scalar.wait_ge(
                                s_pe, pe_base(it - 1) + [4, 12][p - 2])
                    scalar.activation(
                        exp_sb[:, it % 2, p * 1024:(p + 1) * 1024],
                        qk2[p % 2][:, 0:1024],
                        AF.Exp,
                    ).then_inc(s_act, 1)
                    cA += 1
            assert cA == ACT0 + 96

            # phase 3: paired copies with bias
            prj0 = PE0 + 16 * N_ITERS + 4   # s_pe after the tail PVs
            for jt in range(6):
                scalar.wait_ge(s_pe, prj0 + 2 * jt + 2)
                if jt >= 2:
                    scalar.wait_ge([s_out0, s_out1][jt % 2], (jt // 2) * 16)
                scalar.activation(
                    out_sb[jt % 2][:, 0:1024],
                    qk2[jt % 2][:, 0:1024],
                    AF.Identity,
                    bias=projb_sb[:, jt:jt + 1],
                ).then_inc(s_act, 1)
                cA += 1

        block.scalar(_act)

        # ================= vector (DVE) =================
        def _dve(vector):
            vector.memset(krow[:, :], 0x7EF477D5)
            vector.memset(warm[0:1, :], 0.5)
            vector.memset(dummy[0:1, :], 0.0).then_inc(s_init, 1)
            va6 = vaug[:].rearrange("p t (hp blk) -> p t hp blk", blk=192)
            qaug4 = qaug[:].rearrange("p m (h w) -> p m h w", w=32)
            seed_f = seed_sb[:].bitcast(F32)
            cD = 0
            cV = 0
            # phase 1B: odd rel pairs + v prefill copies, in PE order
            for ev, arg in events1b:
                if ev == "rel" and arg % 2 == 1:
                    j = arg
                    bh, bw = rel_banks(j)
                    vector.wait_ge(s_pe, mark_pe_rel[j])
                    vector.tensor_copy(
                        qaug[64:96, :, j * 32:(j + 1) * 32],
                        bh[64:96, 0:384].rearrange("p (m w) -> p m w", w=32),
                    ).then_inc(s_dve, 1)
                    vector.tensor_copy(
                        qaug4[96:128, :, :, j],
                        bw[96:128, 0:384].rearrange("p (m h) -> p m h", h=32),
                    ).then_inc(s_dve, 1)
                    cD += 2
                    assert cD == mark_dve_rel[j]
                elif ev == "v":
                    gv = arg
                    hp, st = gv // 8, gv % 8
                    vector.wait_ge(s_vpe, gv + 1)
                    src = (v_ps if gv % 2 == 0 else bc_ps)[:, 0:128]
                    vector.tensor_copy(
                        va6[:, st, hp, :].rearrange(
                            "p (a c) -> p a c", c=64)[:, 0:3:2, :],
                        src.rearrange("p (m c) -> p m c", c=64),
                    ).then_inc(s_vdve, 1)
                    cV += 1
            assert cD == DVE0 and cV == V_PREFILL

            # phase 2 per iter: recip chain of it, v copies of it+1 (they
            # must precede bccopy(it): bcast(it) at iter it+2 waits them),
            # then bccopy/mul of it
            def vcopies(vit):
                nonlocal cV
                for gv in v_sched[vit]:
                    vector.wait_ge(s_vpe, gv + 1)
                    hp, st = gv // 8, gv % 8
                    vsrc = (v_ps if gv % 2 == 0 else bc_ps)[:, 0:128]
                    vector.tensor_copy(
                        va6[:, st, hp, :].rearrange(
                            "p (a c) -> p a c", c=64)[:, 0:3:2, :],
                        vsrc.rearrange("p (m c) -> p m c", c=64),
                    ).then_inc(s_vdve, 1)
                    cV += 1

            vcopies(0)
            for it, (m, b) in enumerate(iters):
                pb = pe_base(it)
                # denominator rows sit at the opposite half from the v rows
                # (head-parity); the chain runs at the v half reading the
                # denominator half through the PSUM operand (walrus allows
                # cross-base only for PSUM APs). magic-seed + 1 NR.
                dh = 64 if m % 2 == 0 else 0
                vh = 64 - dh
                sv = slice(vh, vh + 64)
                drow = out_ps[it % 2][dh:dh + 64, :]
                vector.wait_ge(s_pe, pv7_inc(it))
                vector.tensor_sub(
                    seed_sb[sv, :], krow[sv, :], drow.bitcast(I32),
                ).then_inc(s_dve, 1)
                vector.scalar_tensor_tensor(
                    t_sb[sv, :], drow, -1.0, seed_f[sv, :],
                    op0=ALU.mult, op1=ALU.mult,
                ).then_inc(s_dve, 1)
                vector.scalar_tensor_tensor(
                    recip_sb[sv, :], t_sb[sv, :], 2.0, seed_f[sv, :],
                    op0=ALU.add, op1=ALU.mult,
                ).then_inc(s_dve, 1)
                vector.tensor_mul(
                    outdT[sv, m // 2, b * 512:(b + 1) * 512],
                    out_ps[it % 2][sv, :],
                    recip_sb[sv, :],
                ).then_inc(s_dve, 1)
                cD += 4
                assert cD == DVE0 + 4 * (it + 1)
                if it + 1 < N_ITERS:
                    vcopies(it + 1)

        block.vector(_dve)

    # clear semaphores so the NEFF is safely re-executable
    nc.reset()
    return nc


def _prep_inputs(x, qkv_w, qkv_b, proj_w, proj_b, rel_pos_h, rel_pos_w):
    """Host-side constant prep shared across cores (everything but xT)."""
    import ml_dtypes
    bf = ml_dtypes.bfloat16
    f32 = np.float32
    wq = qkv_w[0:DIM].astype(f32) * SCALE
    wk = qkv_w[DIM:2 * DIM].astype(f32)
    wv = qkv_w[2 * DIM:3 * DIM].astype(f32)
    def shuf(a):
        # [768, N] -> [128, 6*N]: row p holds all 6 dt-tiles for partition p
        n = a.shape[1]
        return np.ascontiguousarray(
            a.reshape(6, 128, n).transpose(1, 0, 2).reshape(128, 6 * n))

    wqk = np.concatenate(
        [shuf(wq.T), shuf(wk.T)], axis=1).astype(bf).copy()
    wv_t = shuf(wv.T).astype(bf).copy()
    wproj = shuf(proj_w.astype(f32).T).astype(bf).copy()

    qb = qkv_b[0:DIM].astype(f32) * SCALE
    kb = qkv_b[DIM:2 * DIM].astype(f32)
    vb = qkv_b[2 * DIM:3 * DIM].astype(f32)
    qkb = np.concatenate(
        [qb.reshape(6, 128).T, kb.reshape(6, 128).T], axis=1).copy()
    projb_eff = (proj_b.astype(f32) + vb @ proj_w.astype(f32).T)
    projb = projb_eff.reshape(6, 128).T.copy()

    idx = np.arange(H)[:, None] - np.arange(H)[None, :] + (H - 1)
    Rh = rel_pos_h.astype(f32)[idx]
    Rw = rel_pos_w.astype(f32)[idx]
    relh_core = (Rh.transpose(2, 0, 1) / SCALE).reshape(HD, H * H)
    relw_core = (Rw.transpose(2, 0, 1) / SCALE).reshape(HD, W * W)
    # M=64 window tables: relh_j at block j (one pad block after);
    # relw_j at block j+1 (one pad block before)
    relh = np.zeros((HD, 35 * 32), dtype=f32)
    relh[:, 64:64 + 1024] = relh_core    # relh_j at window cols 64:96
    relw = np.zeros((HD, 35 * 32), dtype=f32)
    relw[:, 96:96 + 1024] = relw_core    # relw_j at window cols 96:128

    onehot = np.zeros((HD, S), dtype=f32)
    s = np.arange(S)
    onehot[s // W, s] = 1.0
    onehot[32 + s % W, s] = 1.0
    onescol = np.ones((128, 6 * 64), dtype=bf)

    return dict(wqk=wqk, wv=wv_t, wproj=wproj, relh=relh, relw=relw,
                onehot=onehot, onescol=onescol,
                qkb=qkb, projb=projb)


_CACHED_NC = None


def kernel(x, qkv_w, qkv_b, proj_w, proj_b, rel_pos_h, rel_pos_w,
           trace=False):
    import ml_dtypes
    from concourse.bass_utils import run_bass_kernel_spmd

    global _CACHED_NC
    if _CACHED_NC is None:
        _CACHED_NC = build_nc()
    nc = _CACHED_NC

    consts = _prep_inputs(x, qkv_w, qkv_b, proj_w, proj_b,
                          rel_pos_h, rel_pos_w)
    bf = ml_dtypes.bfloat16
    in_maps = []
    for bb in range(NCORES):
        xTf = np.asarray(x[bb]).reshape(S, DIM).T.reshape(6, 128, S)
        xT = np.ascontiguousarray(
            xTf.transpose(1, 0, 2).reshape(128, 6 * S)).astype(bf)
        in_maps.append({"xT": xT, **consts})

    res = run_bass_kernel_spmd(nc, in_maps, core_ids=list(range(NCORES)),
                               trace=trace)
    outs = []
    for bb in range(NCORES):
        outT = np.asarray(res.results[bb]["outT"]).astype(np.float32)
        outs.append(outT.T.reshape(H, W, DIM))
    full = np.stack(outs, axis=0).astype(np.float32)
    if trace:
        return full, res
    return full

